# revision 7
# baseline (speedup 1.0000x reference)
"""Masked-loss kernel for nn_MLoss_9715216024200 on 8 Trainium2 NeuronCores.

loss = sum(where(y[...,0]>0.5, (y-x)^2 - a*x^2, 0)) + a*sum(x[...,0]^2)
with x,y f32 (256, 10647, 5); output is a f32 scalar.

Sharding: flatten both tensors to cells (5 contiguous f32 each), pad with
256 zero-cells (mathematically neutral: y0=0 -> mask 0, x=0 -> no bg term),
reshape to (8 cores, 128 partitions, 2662 cells).  Each core streams its
13 MiB at the 360 B/ns DMA roofline; every engine runs below the DMA rate
so the post-stream tail is only the last tile's short chain plus the store:

  per tile (c cells, fd=5c elems/partition, bf16 intermediates):
    Pool: m5  = bf16(y0 > 0.5) replicated to 5 features
          xs0 = sqrt(a)*x0 -> dmx[fd:fd+c]
    DVE:  d   = y - x (1x), dm = d*m5 -> dmx[0:fd] (2x),
          msq = m5*xsq (2x)
    ACT:  xsq = x^2 (bf16, no accum)
          acc[2t]   = sum(dmx^2) = sum((m*d)^2) + a*sum(x0^2)   [Square]
          acc[2t+1] = sum(a*msq) = a*sum(m*x^2)                 [Copy]

Tile sizes: mid head tile so compute starts ~3us in, big steady tiles,
then a geometrically shrinking tail; engine choices per tile are knobs.
Host combines: total = sum(acc[even]) - sum(acc[odd]) in f64.
"""
import sys

for _p in ('/opt/trn_rl_repo',):
    if _p in sys.path:
        sys.path.remove(_p)
    sys.path.insert(0, _p)

import os as _os
import numpy as np

B, C, F = 256, 10647, 5
THRESH = 0.5
ALPHA = 0.1
N_CORES = 8
P = 128
CELLS = B * C                      # 2,725,632
CELLS_PER_PART = 2662              # ceil to 8*128*2662 = 2,725,888
PAD_CELLS = N_CORES * P * CELLS_PER_PART - CELLS   # 256
FD = CELLS_PER_PART * F            # 13310 elems per partition per core

_ts = _os.environ.get('TILE_SIZES', '')
TILE_SIZES = ([int(v) for v in _ts.split(',')] if _ts
              else [100, 290, 290, 290, 290, 290, 290, 290, 216, 140, 90, 60, 26])
assert sum(TILE_SIZES) == CELLS_PER_PART, sum(TILE_SIZES)
N_TILES = len(TILE_SIZES)

_env = _os.environ.get


def _iset(name, default=''):
    v = _env(name, default)
    if v == 'all':
        return set(range(N_TILES))
    return set(int(x) for x in v.split(',') if x != '')


XM_TILES = _iset('XM_TILES', 'all')                 # tiles using the xm scheme
M5_ON_DVE = _iset('M5_ON_DVE', 'all')        # default DVE
MSUM_ON_DVE = _iset('MSUM_ON_DVE')           # default ACT Copy-accum
SQ_ON_DVE = _iset('SQ_ON_DVE', str(N_TILES - 1))
XM_ON_DVE = _iset('XM_ON_DVE')               # xm-scheme: xm engine
SQ2_ON_DVE = _iset('SQ2_ON_DVE')             # xm-scheme: sq2 engine
BUFS = [int(v) for v in _env('BUFS', '6,6,4,4').split(',')]
DEFER_K = int(_env('DEFER_K', '1'))

_compiled = None


def _build():
    from contextlib import ExitStack
    import concourse.tile as tile
    from concourse import bacc, mybir

    sqa = float(np.sqrt(ALPHA))

    nc = bacc.Bacc("TRN2", target_bir_lowering=False, debug=False,
                   enable_asserts=True, num_devices=N_CORES)
    x_d = nc.dram_tensor("x", [P, FD], mybir.dt.float32, kind="ExternalInput").ap()
    y_d = nc.dram_tensor("y", [P, FD], mybir.dt.float32, kind="ExternalInput").ap()
    o_d = nc.dram_tensor("o", [P, 2 * N_TILES], mybir.dt.float32,
                         kind="ExternalOutput").ap()

    f32 = mybir.dt.float32
    bf16 = mybir.dt.bfloat16
    Sq = mybir.ActivationFunctionType.Square
    Copy = mybir.ActivationFunctionType.Copy
    Alu = mybir.AluOpType

    with tile.TileContext(nc) as tc, ExitStack() as ctx:
        xp = ctx.enter_context(tc.tile_pool(name="x", bufs=BUFS[0]))
        yp = ctx.enter_context(tc.tile_pool(name="y", bufs=BUFS[1]))
        wp = ctx.enter_context(tc.tile_pool(name="work", bufs=BUFS[2]))
        sp = ctx.enter_context(tc.tile_pool(name="scratch", bufs=BUFS[3]))
        ap_ = ctx.enter_context(tc.tile_pool(name="acc", bufs=1))

        # interleaved acc layout: columns [2t, 2t+1] = (dm-side, masked-x2)
        acc = ap_.tile([P, 2 * N_TILES], f32)

        state = {}

        def primary(t):
            """DMA + mask + d/dm + acc1 square: the per-tile critical chain."""
            cells = TILE_SIZES[t]
            fd = cells * F
            xt = xp.tile([P, fd], f32, tag="xt")
            yt = yp.tile([P, fd], f32, tag="yt")
            off = sum(TILE_SIZES[:t]) * F
            sl = slice(off, off + fd)
            nc.sync.dma_start(yt[:], y_d[:, sl])
            nc.sync.dma_start(xt[:], x_d[:, sl])

            dmx = wp.tile([P, fd + cells], bf16, tag="dmx")

            m5 = wp.tile([P, fd], bf16, tag="m5")
            y0b = yt[:, 0::F].unsqueeze(2).broadcast_to((P, cells, F))
            m5_eng = nc.vector if t in M5_ON_DVE else nc.gpsimd
            m5_eng.tensor_scalar(
                m5[:].rearrange("p (k f) -> p k f", f=F), y0b,
                THRESH, None, op0=Alu.is_gt)

            # Pool: xs0 = sqrt(a)*x0 into the tail slice of dmx
            nc.gpsimd.tensor_scalar(dmx[:, fd:fd + cells], xt[:, 0::F],
                                    sqa, None, op0=Alu.mult)

            # DVE: d = y - x (bf16 out), dm = d*m5 (2x)
            dt_ = wp.tile([P, fd], bf16, tag="d")
            nc.vector.tensor_tensor(dt_[:], yt[:], xt[:], op=Alu.subtract)
            nc.vector.tensor_tensor(dmx[:, 0:fd], dt_[:], m5[:], op=Alu.mult)

            # acc1: sum(dmx^2)
            sq = sp.tile([P, fd + cells], bf16, tag="sq")
            if t in SQ_ON_DVE:
                nc.vector.scalar_tensor_tensor(
                    sq[:], dmx[:], 1.0, dmx[:], op0=Alu.mult, op1=Alu.mult,
                    accum_out=acc[:, 2 * t:2 * t + 1])
            else:
                nc.scalar.activation(sq[:], dmx[:], Sq,
                                     accum_out=acc[:, 2 * t:2 * t + 1])
            state[t] = (xt, m5)

        def secondary(t):
            """acc2 = a*sum(m*x^2): deferred so it never blocks primaries."""
            cells = TILE_SIZES[t]
            fd = cells * F
            xt, m5 = state.pop(t)
            if t in XM_TILES:
                xmt = wp.tile([P, fd], bf16, tag="xm")
                xm_eng = nc.vector if t in XM_ON_DVE else nc.gpsimd
                xm_eng.tensor_tensor(xmt[:], xt[:], m5[:], op=Alu.mult)
                sq2 = sp.tile([P, fd], bf16, tag="sq2")
                if t in SQ2_ON_DVE:
                    nc.vector.scalar_tensor_tensor(
                        sq2[:], xmt[:], ALPHA, xmt[:], op0=Alu.mult, op1=Alu.mult,
                        accum_out=acc[:, 2 * t + 1:2 * t + 2])
                else:
                    nc.scalar.activation(sq2[:], xmt[:], Sq, scale=sqa,
                                         accum_out=acc[:, 2 * t + 1:2 * t + 2])
            else:
                xsq = wp.tile([P, fd], bf16, tag="xsq")
                nc.scalar.activation(xsq[:], xt[:], Sq)
                msq = wp.tile([P, fd], bf16, tag="msq")
                nc.vector.tensor_tensor(msq[:], m5[:], xsq[:], op=Alu.mult)
                msum = sp.tile([P, fd], bf16, tag="msum")
                if t in MSUM_ON_DVE:
                    nc.vector.tensor_scalar(
                        msum[:], msq[:], ALPHA, None, op0=Alu.mult,
                        accum_out=acc[:, 2 * t + 1:2 * t + 2])
                else:
                    nc.scalar.activation(msum[:], msq[:], Copy, scale=ALPHA,
                                         accum_out=acc[:, 2 * t + 1:2 * t + 2])

        for t in range(N_TILES):
            primary(t)
            if t - DEFER_K >= 0:
                secondary(t - DEFER_K)
        for t in range(max(0, N_TILES - DEFER_K), N_TILES):
            secondary(t)

        nc.sync.dma_start(o_d[:], acc[:])

    nc.compile()
    return nc


def _shard(a: np.ndarray) -> list[np.ndarray]:
    flat = a.reshape(-1)
    pad = np.zeros(PAD_CELLS * F, dtype=a.dtype)
    flat = np.concatenate([flat, pad])
    per_core = flat.reshape(N_CORES, P, FD)
    return [np.ascontiguousarray(per_core[i]) for i in range(N_CORES)]


def kernel(x: np.ndarray, y: np.ndarray) -> np.ndarray:
    global _compiled
    if _compiled is None:
        _compiled = _build()
    nc = _compiled

    from concourse.bass_utils import run_bass_kernel_spmd

    xs = _shard(np.asarray(x, dtype=np.float32))
    ys = _shard(np.asarray(y, dtype=np.float32))
    in_maps = [{"x": xs[i], "y": ys[i]} for i in range(N_CORES)]
    res = run_bass_kernel_spmd(nc, in_maps, core_ids=list(range(N_CORES)))

    total = np.float64(0.0)
    for r in res.results:
        o = r["o"].astype(np.float64)
        total += o[:, 0::2].sum()
        total -= o[:, 1::2].sum()
    return np.float32(total)


# revision 10
# speedup vs baseline: 1.0710x; 1.0710x over previous
"""Masked-loss kernel for nn_MLoss_9715216024200 on 8 Trainium2 NeuronCores.

loss = sum(where(y[...,0]>0.5, (y-x)^2 - a*x^2, 0)) + a*sum(x[...,0]^2)
with x,y f32 (256, 10647, 5); output is a f32 scalar.

Sharding: flatten both tensors to cells (5 contiguous f32 each), pad with
256 zero-cells (mathematically neutral: y0=0 -> mask 0, x=0 -> no bg term),
reshape to (8 cores, 128 partitions, 2662 cells).  Each core streams its
13 MiB at the 360 B/ns DMA roofline; every engine runs below the DMA rate
so the post-stream tail is only the last tile's short chain plus the store:

  per tile (c cells, fd=5c elems/partition, bf16 intermediates):
    Pool: m5  = bf16(y0 > 0.5) replicated to 5 features
          xs0 = sqrt(a)*x0 -> dmx[fd:fd+c]
    DVE:  d   = y - x (1x), dm = d*m5 -> dmx[0:fd] (2x),
          msq = m5*xsq (2x)
    ACT:  xsq = x^2 (bf16, no accum)
          acc[2t]   = sum(dmx^2) = sum((m*d)^2) + a*sum(x0^2)   [Square]
          acc[2t+1] = sum(a*msq) = a*sum(m*x^2)                 [Copy]

Tile sizes: mid head tile so compute starts ~3us in, big steady tiles,
then a geometrically shrinking tail; engine choices per tile are knobs.
Host combines: total = sum(acc[even]) - sum(acc[odd]) in f64.
"""
import sys

for _p in ('/opt/trn_rl_repo',):
    if _p in sys.path:
        sys.path.remove(_p)
    sys.path.insert(0, _p)

import os as _os
import numpy as np

B, C, F = 256, 10647, 5
THRESH = 0.5
ALPHA = 0.1
N_CORES = 8
P = 128
CELLS = B * C                      # 2,725,632
CELLS_PER_PART = 2662              # ceil to 8*128*2662 = 2,725,888
PAD_CELLS = N_CORES * P * CELLS_PER_PART - CELLS   # 256
FD = CELLS_PER_PART * F            # 13310 elems per partition per core

_ts = _os.environ.get('TILE_SIZES', '')
TILE_SIZES = ([int(v) for v in _ts.split(',')] if _ts
              else [100, 290, 290, 290, 290, 290, 290, 290, 216, 140, 90, 60, 26])
assert sum(TILE_SIZES) == CELLS_PER_PART, sum(TILE_SIZES)
N_TILES = len(TILE_SIZES)

_env = _os.environ.get


def _iset(name, default=''):
    v = _env(name, default)
    if v == 'all':
        return set(range(N_TILES))
    return set(int(x) for x in v.split(',') if x != '')


XM_TILES = _iset('XM_TILES', 'all')                 # tiles using the xm scheme
M5_ON_DVE = _iset('M5_ON_DVE', 'all')        # default DVE
MSUM_ON_DVE = _iset('MSUM_ON_DVE')           # default ACT Copy-accum
SQ_ON_DVE = _iset('SQ_ON_DVE', str(N_TILES - 1))
XM_ON_DVE = _iset('XM_ON_DVE')               # xm-scheme: xm engine
SQ2_ON_DVE = _iset('SQ2_ON_DVE')             # xm-scheme: sq2 engine
BUFS = [int(v) for v in _env('BUFS', '6,6,4,4').split(',')]
DEFER_K = int(_env('DEFER_K', '1'))
PERSIST = _env('PERSIST', '1') == '1'

_compiled = None


def _build():
    from contextlib import ExitStack
    import concourse.tile as tile
    from concourse import bacc, mybir

    sqa = float(np.sqrt(ALPHA))

    nc = bacc.Bacc("TRN2", target_bir_lowering=False, debug=False,
                   enable_asserts=True, num_devices=N_CORES)
    x_d = nc.dram_tensor("x", [P, FD], mybir.dt.float32, kind="ExternalInput").ap()
    y_d = nc.dram_tensor("y", [P, FD], mybir.dt.float32, kind="ExternalInput").ap()
    o_d = nc.dram_tensor("o", [P, 2 * N_TILES], mybir.dt.float32,
                         kind="ExternalOutput").ap()

    f32 = mybir.dt.float32
    bf16 = mybir.dt.bfloat16
    Sq = mybir.ActivationFunctionType.Square
    Copy = mybir.ActivationFunctionType.Copy
    Alu = mybir.AluOpType

    with tile.TileContext(nc) as tc, ExitStack() as ctx:
        xp = ctx.enter_context(tc.tile_pool(name="x", bufs=BUFS[0]))
        yp = ctx.enter_context(tc.tile_pool(name="y", bufs=BUFS[1]))
        wp = ctx.enter_context(tc.tile_pool(name="work", bufs=BUFS[2]))
        sp = ctx.enter_context(tc.tile_pool(name="scratch", bufs=BUFS[3]))
        ap_ = ctx.enter_context(tc.tile_pool(name="acc", bufs=1))

        # interleaved acc layout: columns [2t, 2t+1] = (dm-side, masked-x2)
        acc = ap_.tile([P, 2 * N_TILES], f32)

        state = {}

        def primary(t):
            """DMA + mask + d/dm + acc1 square: the per-tile critical chain."""
            cells = TILE_SIZES[t]
            fd = cells * F
            if PERSIST:
                # per-tile dedicated buffers: x and m5 stay alive all stream,
                # so xm/sq2 can be scheduled whenever Pool/ACT have slack
                xt = xp.tile([P, fd], f32, tag=f"xt{t}", bufs=1)
            else:
                xt = xp.tile([P, fd], f32, tag="xt")
            yt = yp.tile([P, fd], f32, tag="yt")
            off = sum(TILE_SIZES[:t]) * F
            sl = slice(off, off + fd)
            nc.sync.dma_start(yt[:], y_d[:, sl])
            nc.sync.dma_start(xt[:], x_d[:, sl])

            dmx = wp.tile([P, fd + cells], bf16, tag="dmx")

            if PERSIST:
                m5 = wp.tile([P, fd], bf16, tag=f"m5{t}", bufs=1)
            else:
                m5 = wp.tile([P, fd], bf16, tag="m5")
            y0b = yt[:, 0::F].unsqueeze(2).broadcast_to((P, cells, F))
            m5_eng = nc.vector if t in M5_ON_DVE else nc.gpsimd
            m5_eng.tensor_scalar(
                m5[:].rearrange("p (k f) -> p k f", f=F), y0b,
                THRESH, None, op0=Alu.is_gt)

            # Pool: xs0 = sqrt(a)*x0 into the tail slice of dmx
            nc.gpsimd.tensor_scalar(dmx[:, fd:fd + cells], xt[:, 0::F],
                                    sqa, None, op0=Alu.mult)

            # DVE: d = y - x (bf16 out), dm = d*m5 (2x)
            dt_ = wp.tile([P, fd], bf16, tag="d")
            nc.vector.tensor_tensor(dt_[:], yt[:], xt[:], op=Alu.subtract)
            nc.vector.tensor_tensor(dmx[:, 0:fd], dt_[:], m5[:], op=Alu.mult)

            # acc1: sum(dmx^2)
            sq = sp.tile([P, fd + cells], bf16, tag="sq")
            if t in SQ_ON_DVE:
                nc.vector.scalar_tensor_tensor(
                    sq[:], dmx[:], 1.0, dmx[:], op0=Alu.mult, op1=Alu.mult,
                    accum_out=acc[:, 2 * t:2 * t + 1])
            else:
                nc.scalar.activation(sq[:], dmx[:], Sq,
                                     accum_out=acc[:, 2 * t:2 * t + 1])
            state[t] = (xt, m5)

        def secondary(t):
            """acc2 = a*sum(m*x^2): deferred so it never blocks primaries."""
            cells = TILE_SIZES[t]
            fd = cells * F
            xt, m5 = state.pop(t)
            if t in XM_TILES:
                xmt = wp.tile([P, fd], bf16, tag="xm")
                xm_eng = nc.vector if t in XM_ON_DVE else nc.gpsimd
                xm_eng.tensor_tensor(xmt[:], xt[:], m5[:], op=Alu.mult)
                sq2 = sp.tile([P, fd], bf16, tag="sq2")
                if t in SQ2_ON_DVE:
                    nc.vector.scalar_tensor_tensor(
                        sq2[:], xmt[:], ALPHA, xmt[:], op0=Alu.mult, op1=Alu.mult,
                        accum_out=acc[:, 2 * t + 1:2 * t + 2])
                else:
                    nc.scalar.activation(sq2[:], xmt[:], Sq, scale=sqa,
                                         accum_out=acc[:, 2 * t + 1:2 * t + 2])
            else:
                xsq = wp.tile([P, fd], bf16, tag="xsq")
                nc.scalar.activation(xsq[:], xt[:], Sq)
                msq = wp.tile([P, fd], bf16, tag="msq")
                nc.vector.tensor_tensor(msq[:], m5[:], xsq[:], op=Alu.mult)
                msum = sp.tile([P, fd], bf16, tag="msum")
                if t in MSUM_ON_DVE:
                    nc.vector.tensor_scalar(
                        msum[:], msq[:], ALPHA, None, op0=Alu.mult,
                        accum_out=acc[:, 2 * t + 1:2 * t + 2])
                else:
                    nc.scalar.activation(msum[:], msq[:], Copy, scale=ALPHA,
                                         accum_out=acc[:, 2 * t + 1:2 * t + 2])

        for t in range(N_TILES):
            primary(t)
            if t - DEFER_K >= 0:
                secondary(t - DEFER_K)
        for t in range(max(0, N_TILES - DEFER_K), N_TILES):
            secondary(t)

        nc.sync.dma_start(o_d[:], acc[:])

    nc.compile()
    return nc


def _shard(a: np.ndarray) -> list[np.ndarray]:
    flat = a.reshape(-1)
    pad = np.zeros(PAD_CELLS * F, dtype=a.dtype)
    flat = np.concatenate([flat, pad])
    per_core = flat.reshape(N_CORES, P, FD)
    return [np.ascontiguousarray(per_core[i]) for i in range(N_CORES)]


def kernel(x: np.ndarray, y: np.ndarray) -> np.ndarray:
    global _compiled
    if _compiled is None:
        _compiled = _build()
    nc = _compiled

    from concourse.bass_utils import run_bass_kernel_spmd

    xs = _shard(np.asarray(x, dtype=np.float32))
    ys = _shard(np.asarray(y, dtype=np.float32))
    in_maps = [{"x": xs[i], "y": ys[i]} for i in range(N_CORES)]
    res = run_bass_kernel_spmd(nc, in_maps, core_ids=list(range(N_CORES)))

    total = np.float64(0.0)
    for r in res.results:
        o = r["o"].astype(np.float64)
        total += o[:, 0::2].sum()
        total -= o[:, 1::2].sum()
    return np.float32(total)
